# revision 11
# baseline (speedup 1.0000x reference)
"""Analytic Gaussian VP score on 8 TRN2 NeuronCores.

Math: per sample i, score_i = -Sigma_i^{-1} (x_i - a_i*mean0) with
Sigma_i = a_i^2*cov0 + s_i^2*I.  All Sigma_i are shifted/scaled versions of
one shared matrix, so instead of 128 per-sample Choleskys we apply a
per-sample degree-NK Chebyshev polynomial of cov0:

    score_i = -sum_k c_{i,k} T_k(Mt) u_i,   Mt = (cov0 - MID*I)/HALF

The coefficients c_{i,k} are computed ON DEVICE from t_i (ScalarE exp +
DVE + small PE matmuls).  The T_k are generated four at a time: with
T4 = T_4(Mt) built once on device (two matrix squarings), the identity
T_{k+4} = 2*T4*T_k - T_{k-4} advances four independent chains per batched
matmul, so the TensorEngine runs [128,128]-weight matmuls with a 64-wide
moving operand (the 4*16 chain/sample columns) instead of a pathological
16-wide one.  A basis scaling gamma = HALF/2 folds all affine constants
into the stored matrices/coefficients; spectral bounds [L, U] hold
structurally (cov0 = PSD + 0.1*I, Marchenko-Pastur bulk < 4.1).

Sharding: pure data parallelism - mean0/cov0 replicated, the 128 (t, x)
pairs split 16 per core, no collectives.  State is kept transposed
([feature, (chunk, chain, sample)] = [128 x 256]) so the matmul is
cov0-stationary (symmetric blocks, no transposes anywhere) and all
elementwise ops run on fully-occupied 128-partition tiles.  float32r
(fp32 storage, fp22 multiply) everywhere on the matmul path; validated
rel err ~1e-3 vs the 2e-2 gate.
"""

import numpy as np

try:
    import concourse.bass as bass
except ImportError:  # fresh grading dir: point at the staged repo
    import sys

    for _p in ("/opt/trn_rl_repo", "/root/.axon_site/_ro/trn_rl_repo"):
        if _p not in sys.path:
            sys.path.insert(0, _p)
    import concourse.bass as bass

from contextlib import ExitStack

import concourse.tile as tile
from concourse import bacc, mybir
from concourse.bass_utils import run_bass_kernel_spmd

F32 = mybir.dt.float32
F32R = mybir.dt.float32r
BF16 = mybir.dt.bfloat16
AL = mybir.AluOpType
AX = mybir.AxisListType

B, D = 128, 512
NCORES = 8
BLOC = B // NCORES  # 16 samples per core
KC = D // 128  # 4 partition chunks of the feature dim
NCH = 4  # Chebyshev chains advanced per step
W = NCH * BLOC  # 64: moving-operand width of the main matmuls

# Chebyshev setup (input-independent constants)
L_BND, U_BND = 0.0995, 4.10
NN = 64  # interpolation nodes
NK = 23  # polynomial degree; NK+1 = 24 coefficients = 4 chains x 6 steps
NSTEP = (NK + 1) // 4 - 1  # T4-steps (first one special)
MID = (U_BND + L_BND) / 2.0
HALF = (U_BND - L_BND) / 2.0
GAMMA = HALF / 2.0  # basis scaling; makes the matmuls consume raw matrices
G2 = GAMMA * GAMMA
G8 = GAMMA**8
HALF2 = HALF * HALF
T4DIAG = HALF**4 / 8.0
BETA_MIN, BETA_MAX = 0.1, 20.0


def _host_constants():
    j = np.arange(NN)
    th = np.pi * (j + 0.5) / NN
    lam = (MID + HALF * np.cos(th)).astype(np.float32).reshape(NN, 1)
    k = np.arange(NK + 1)
    dm = (2.0 / NN) * np.cos(k[None, :] * th[:, None])
    dm[:, 0] *= 0.5
    dm = (-dm) * (1.0 / np.float64(GAMMA)) ** k[None, :]  # fold -1, gamma^-k
    dmat = dm.astype(np.float32)
    ones1 = np.ones((1, 128), np.float32)
    ones64 = np.ones((NN, 128), np.float32)
    eye = np.eye(128, dtype=np.float32)
    return lam, dmat, ones1, ones64, eye


def _build_nc():
    nc = bacc.Bacc()
    t_row = nc.declare_dram_parameter("t_row", [1, BLOC], F32, isOutput=False)
    xT = nc.declare_dram_parameter("xT", [D, BLOC], F32, isOutput=False)
    mean_pk = nc.declare_dram_parameter("mean_pk", [128, KC], F32, isOutput=False)
    cov0 = nc.declare_dram_parameter("cov0", [D, D], F32R, isOutput=False)
    lam = nc.declare_dram_parameter("lam", [NN, 1], F32, isOutput=False)
    dmat = nc.declare_dram_parameter("dmat", [NN, NK + 1], F32, isOutput=False)
    ones1 = nc.declare_dram_parameter("ones1", [1, 128], F32, isOutput=False)
    ones64 = nc.declare_dram_parameter("ones64", [NN, 128], F32, isOutput=False)
    eye = nc.declare_dram_parameter("eye", [128, 128], F32, isOutput=False)
    outT = nc.declare_dram_parameter("outT", [D, BLOC], F32, isOutput=True)

    with ExitStack() as ctx:
        tc = ctx.enter_context(tile.TileContext(nc))
        const = ctx.enter_context(tc.tile_pool(name="const", bufs=1))
        state = ctx.enter_context(tc.tile_pool(name="state", bufs=1))
        work = ctx.enter_context(tc.tile_pool(name="work", bufs=2))
        ps_mm = ctx.enter_context(tc.tile_pool(name="ps_mm", bufs=2, space="PSUM"))
        ps_mv = ctx.enter_context(tc.tile_pool(name="ps_mv", bufs=2, space="PSUM"))
        ps_one = ctx.enter_context(tc.tile_pool(name="ps_one", bufs=1, space="PSUM"))

        # ---- PE warm-up: memset a scratch tile (no DMA dependency), then
        # dummy fp32 matmuls so the HAM clock gate is released before the
        # real matmuls start
        warm_sb = const.tile([128, 128], F32, tag="warm_sb")
        nc.gpsimd.memset(warm_sb[:], 1.0)
        warm_ps = ps_one.tile([128, 128], F32, tag="warm")
        for _ in range(12):
            nc.tensor.matmul(warm_ps[:], warm_sb[:], warm_sb[:])
        # ---- loads (spread across engine DGE queues) ----
        eye_sb = const.tile([128, 128], F32, tag="eye")
        nc.gpsimd.dma_start(eye_sb[:], eye[:])
        cov_sb = []
        dma_engs = [nc.sync, nc.scalar, nc.gpsimd]
        for kc in range(KC):
            ct = const.tile([128, D], F32R, tag=f"cov{kc}", name=f"cov{kc}")
            for h in range(2):
                dma_engs[(2 * kc + h) % 3].dma_start(
                    ct[:, h * 256 : (h + 1) * 256],
                    cov0[kc * 128 : (kc + 1) * 128, h * 256 : (h + 1) * 256],
                )
            cov_sb.append(ct)
        xhat = state.tile([128, KC * BLOC], F32, tag="xhat")
        nc.scalar.dma_start(
            xhat[:].rearrange("p (k i) -> p k i", k=KC),
            xT[:].rearrange("(k p) i -> p k i", p=128),
        )
        mhat = const.tile([128, KC], F32, tag="mhat")
        nc.scalar.dma_start(mhat[:], mean_pk[:])
        trow = const.tile([1, BLOC], F32, tag="trow")
        nc.gpsimd.dma_start(trow[:], t_row[:])
        lam_sb = const.tile([NN, 1], F32, tag="lam")
        nc.gpsimd.dma_start(lam_sb[:], lam[:])
        dmat_sb = const.tile([NN, NK + 1], F32, tag="dmat")
        nc.sync.dma_start(dmat_sb[:], dmat[:])
        ones1_sb = const.tile([1, 128], F32, tag="ones1")
        nc.scalar.dma_start(ones1_sb[:], ones1[:])
        ones64_sb = const.tile([NN, 128], F32, tag="ones64")
        nc.sync.dma_start(ones64_sb[:], ones64[:])

        # ---- per-sample scalars from t ----
        u9 = const.tile([1, BLOC], F32, tag="u9")
        nc.vector.tensor_scalar(u9[:], trow[:], 9.95, 0.1, AL.mult, AL.add)
        ib = const.tile([1, BLOC], F32, tag="ib")
        nc.vector.tensor_mul(ib[:], u9[:], trow[:])
        a_row = const.tile([1, BLOC], F32, tag="a_row")
        nc.scalar.activation(
            a_row[:], ib[:], mybir.ActivationFunctionType.Exp, scale=-0.5
        )
        abc = const.tile([1, 3 * BLOC], F32, tag="abc")  # [a | a^2 | s^2]
        nc.vector.tensor_copy(abc[:, 0:BLOC], a_row[:])
        nc.vector.tensor_mul(abc[:, BLOC : 2 * BLOC], a_row[:], a_row[:])
        nc.vector.tensor_scalar(
            abc[:, 2 * BLOC :], abc[:, BLOC : 2 * BLOC], -1.0, 1.0, AL.mult, AL.add
        )
        nc.vector.tensor_scalar_max(abc[:, 2 * BLOC :], abc[:, 2 * BLOC :], 1e-12)

        # broadcast [a | a^2 | s^2] down all 128 partitions via a K=1 matmul
        rep_ps = ps_one.tile([128, 3 * BLOC], F32, tag="rep")
        nc.tensor.matmul(rep_ps[:], ones1_sb[:], abc[:])
        rep = const.tile([128, 3 * BLOC], F32, tag="rep_sb")
        nc.scalar.copy(rep[:], rep_ps[:])
        a_rep = rep[:, 0:BLOC]
        a2_rep = rep[:, BLOC : 2 * BLOC]
        s2_rep = rep[:, 2 * BLOC : 3 * BLOC]

        # ---- Chebyshev coefficients on device ----
        q = const.tile([NN, BLOC], F32, tag="q")
        nc.vector.scalar_tensor_tensor(
            q[:], a2_rep[0:NN, :], lam_sb[:, 0:1], s2_rep[0:NN, :], AL.mult, AL.add
        )
        fhat = const.tile([NN, BLOC], F32, tag="fhat")
        nc.vector.reciprocal(fhat[:], q[:])
        rhs_t = const.tile([NN, (NK + 1) * BLOC], F32, tag="rhs_t")
        nc.vector.tensor_mul(
            rhs_t[:].rearrange("p (k i) -> p k i", k=NK + 1),
            fhat[:].unsqueeze(1).broadcast_to((NN, NK + 1, BLOC)),
            dmat_sb[:].unsqueeze(2).broadcast_to((NN, NK + 1, BLOC)),
        )
        c_ps = ps_one.tile([128, (NK + 1) * BLOC], F32, tag="cps")
        nc.tensor.matmul(c_ps[:], ones64_sb[:], rhs_t[:])
        c_sb = const.tile([128, (NK + 1) * BLOC], F32, tag="c_sb")
        nc.scalar.copy(c_sb[:], c_ps[:])

        def cstep(s):
            """coefficients for step s: [128, (chain, sample)] bcast over kc."""
            return (
                c_sb[:, s * W : (s + 1) * W]
                .unsqueeze(1)
                .broadcast_to((128, KC, W))
            )

        # ---- Btil = C^2 - 2*MID*C + MID^2*I  (stored as 4 row-chunk tiles) --
        btil = [
            const.tile([128, D], F32R, tag=f"btil{r}", name=f"btil{r}")
            for r in range(KC)
        ]
        for r in range(KC):
            c2 = ps_mm.tile([128, D], F32, tag="mm", name="c2")
            for kc in range(KC):
                nc.tensor.matmul(
                    c2[:],
                    cov_sb[kc][:, r * 128 : (r + 1) * 128],
                    cov_sb[kc][:],
                    start=(kc == 0),
                    stop=(kc == KC - 1),
                )
            nc.vector.scalar_tensor_tensor(
                btil[r][:], cov_sb[r][:], -2.0 * MID, c2[:], AL.mult, AL.add
            )
            nc.vector.scalar_tensor_tensor(
                btil[r][:, r * 128 : (r + 1) * 128],
                eye_sb[:],
                MID * MID,
                btil[r][:, r * 128 : (r + 1) * 128],
                AL.mult,
                AL.add,
            )

        # ---- T4h = Btil^2 - HALF^2*Btil + (HALF^4/8)*I  (= 2*gamma^4*T_4(Mt))
        t4 = [
            const.tile([128, D], F32R, tag=f"t4{r}", name=f"t4{r}")
            for r in range(KC)
        ]
        for r in range(KC):
            b2 = ps_mm.tile([128, D], F32, tag="mm", name="b2")
            for kc in range(KC):
                nc.tensor.matmul(
                    b2[:],
                    btil[kc][:, r * 128 : (r + 1) * 128],
                    btil[kc][:],
                    start=(kc == 0),
                    stop=(kc == KC - 1),
                )
            nc.vector.scalar_tensor_tensor(
                t4[r][:], btil[r][:], -HALF2, b2[:], AL.mult, AL.add
            )
            nc.vector.scalar_tensor_tensor(
                t4[r][:, r * 128 : (r + 1) * 128],
                eye_sb[:],
                T4DIAG,
                t4[r][:, r * 128 : (r + 1) * 128],
                AL.mult,
                AL.add,
            )

        # ---- split T4h into bf16 hi + lo (weights at ~2^-16 precision) ----
        thi = [
            const.tile([128, D], BF16, tag=f"thi{r}", name=f"thi{r}")
            for r in range(KC)
        ]
        tlo = [
            const.tile([128, D], BF16, tag=f"tlo{r}", name=f"tlo{r}")
            for r in range(KC)
        ]
        for r in range(KC):
            nc.scalar.copy(thi[r][:], t4[r][:].bitcast(F32))
            nc.vector.tensor_sub(tlo[r][:], t4[r][:].bitcast(F32), thi[r][:])

        # ---- state: X [128, (kc, chain, sample)], bf16 ----
        xs = [
            state.tile([128, KC * W], BF16, tag=f"X{i}", name=f"X{i}")
            for i in range(3)
        ]
        y01 = state.tile([128, KC * 2 * BLOC], F32R, tag="y01")  # f32 Y0,Y1
        acc = state.tile([128, KC * BLOC], F32, tag="acc")

        def chain(st, r):
            """[128, kc, BLOC] view of chain r of state tile st."""
            return st[:].rearrange("p (k r i) -> p k r i", k=KC, r=NCH)[:, :, r, :]

        def v3(ap):
            return ap.rearrange("p (k i) -> p k i", k=KC)

        def matvec(dst_ps, mats, src):
            """dst_ps[:, mc*16:...] += mats^T-block @ src ([128,16] slices)."""
            for mc in range(KC):
                for kc in range(KC):
                    nc.tensor.matmul(
                        dst_ps[:, mc * BLOC : (mc + 1) * BLOC],
                        mats[kc][:, mc * 128 : (mc + 1) * 128],
                        src[:, kc, :].bitcast(F32R),
                        start=(kc == 0),
                        stop=(kc == KC - 1),
                    )

        def matstep(dst_ps, st):
            """dst_ps[:, mc*W:...] += T4h-block @ st, bf16 hi/lo weights."""
            for mc in range(KC):
                first = True
                for kc in range(KC):
                    for mats in (thi, tlo):
                        nc.tensor.matmul(
                            dst_ps[:, mc * W : (mc + 1) * W],
                            mats[kc][:, mc * 128 : (mc + 1) * 128],
                            st[:, kc * W : (kc + 1) * W],
                            start=first,
                            stop=(kc == KC - 1 and mats is tlo),
                        )
                        first = False

        # ---- init: Y0 = u = x - a*mean0 (f32 scratch y01 + bf16 chain) ----
        x0 = xs[0]
        y01v = y01[:].rearrange("p (k r i) -> p k r i", k=KC, r=2)
        w1 = work.tile([128, KC * BLOC], F32, tag="w1")
        nc.vector.tensor_mul(
            v3(w1[:]),
            a_rep.unsqueeze(1).broadcast_to((128, KC, BLOC)),
            mhat[:].unsqueeze(2).broadcast_to((128, KC, BLOC)),
        )
        nc.vector.tensor_sub(y01v[:, :, 0, :], v3(xhat[:]), v3(w1[:]))
        nc.scalar.copy(chain(x0, 0), y01v[:, :, 0, :])

        # Y1 = 0.5*C@Y0 - (MID/2)*Y0
        p1 = ps_mv.tile([128, KC * BLOC], F32, tag="pmv", name="p1")
        matvec(p1, cov_sb, y01v[:, :, 0, :])
        w2 = work.tile([128, KC * BLOC], F32, tag="w2")
        nc.vector.tensor_scalar_mul(v3(w2[:]), y01v[:, :, 0, :], -MID / 2.0)
        nc.vector.scalar_tensor_tensor(
            y01v[:, :, 1, :], v3(p1[:]), 0.5, v3(w2[:]), AL.mult, AL.add
        )
        nc.scalar.copy(chain(x0, 1), y01v[:, :, 1, :])

        # one T2-step: P = Btil@[Y0|Y1]; Y2 = 0.5*P0 - g2*Y0; Y3 = P1 - 3*g2*Y1
        p2 = ps_mv.tile([128, KC * 2 * BLOC], F32, tag="pmv", name="p2")
        for mc in range(KC):
            for kc in range(KC):
                nc.tensor.matmul(
                    p2[:, mc * 2 * BLOC : (mc + 1) * 2 * BLOC],
                    btil[kc][:, mc * 128 : (mc + 1) * 128],
                    y01[:, kc * 2 * BLOC : (kc + 1) * 2 * BLOC].bitcast(F32R),
                    start=(kc == 0),
                    stop=(kc == KC - 1),
                )
        p2v = p2[:].rearrange("p (k r i) -> p k r i", k=KC, r=2)
        w3 = work.tile([128, KC * BLOC], F32, tag="w2", name="w3")
        nc.vector.tensor_scalar_mul(v3(w3[:]), y01v[:, :, 0, :], -G2)
        nc.vector.scalar_tensor_tensor(
            chain(x0, 2), p2v[:, :, 0, :], 0.5, v3(w3[:]), AL.mult, AL.add
        )
        nc.vector.scalar_tensor_tensor(
            chain(x0, 3), y01v[:, :, 1, :], -3.0 * G2, p2v[:, :, 1, :],
            AL.mult, AL.add,
        )

        def acc_step(st, s, first=False):
            """acc[:, kc, i] += sum_r c[4s+r, i] * st[:, kc, r, i]."""
            mt = work.tile([128, KC * W], F32, tag="mt")
            nc.vector.tensor_mul(
                mt[:].rearrange("p (k w) -> p k w", k=KC),
                st[:].rearrange("p (k w) -> p k w", k=KC),
                cstep(s),
            )
            red = mt[:].rearrange("p (k r i) -> p k i r", k=KC, r=NCH)
            if first:
                nc.vector.tensor_reduce(v3(acc[:]), red, AX.X, AL.add)
            else:
                rt = work.tile([128, KC * BLOC], F32, tag="rt")
                nc.vector.tensor_reduce(v3(rt[:]), red, AX.X, AL.add)
                nc.vector.tensor_add(acc[:], acc[:], rt[:])

        acc_step(x0, 0, first=True)

        # ---- step 1 (special): X1[r] = T4h@X0[r] - g^{2r}*Y_{4-r}; r=0 halved
        x1 = xs[1]
        z = ps_mm.tile([128, KC * W], F32, tag="mm", name="z1")
        matstep(z, x0[:])
        zv = z[:].rearrange("p (k r i) -> p k r i", k=KC, r=NCH)
        nc.vector.tensor_scalar_mul(chain(x1, 0), zv[:, :, 0, :], 0.5)
        for r in (1, 2, 3):
            nc.vector.scalar_tensor_tensor(
                chain(x1, r),
                chain(x0, NCH - r),
                -(GAMMA ** (2 * r)),
                zv[:, :, r, :],
                AL.mult,
                AL.add,
            )
        acc_step(x1, 1)

        # ---- steps 2..NSTEP: Xn = T4h@Xc - gamma^8*Xp ----
        xp, xc, xn = xs
        outv = outT[:].rearrange("(k p) i -> p k i", p=128)
        dma_engs2 = [nc.sync, nc.scalar, nc.gpsimd, nc.sync]
        for s in range(2, NSTEP + 1):
            P = ps_mm.tile([128, KC * W], F32, tag="mm", name=f"P{s}")
            matstep(P, xc[:])
            last = s == NSTEP
            for kc in range(KC):  # chunked: next step's kc=0 mm starts early
                sl = slice(kc * W, (kc + 1) * W)
                nc.vector.scalar_tensor_tensor(
                    xn[:, sl], xp[:, sl], -G8, P[:, sl], AL.mult, AL.add
                )
                if last:
                    # finish acc for this chunk and ship it out
                    mt = work.tile([128, W], F32, tag="mtc", name=f"mtc{kc}")
                    nc.vector.tensor_mul(
                        mt[:], xn[:, sl], c_sb[:, s * W : (s + 1) * W]
                    )
                    rt = work.tile([128, BLOC], F32, tag="rtc", name=f"rtc{kc}")
                    nc.vector.tensor_reduce(
                        rt[:].unsqueeze(1),
                        mt[:].rearrange("p (r i) -> p i r", r=NCH),
                        AX.X,
                        AL.add,
                    )
                    nc.vector.tensor_add(
                        acc[:, kc * BLOC : (kc + 1) * BLOC],
                        acc[:, kc * BLOC : (kc + 1) * BLOC],
                        rt[:],
                    )
                    dma_engs2[kc].dma_start(
                        outv[:, kc, :], acc[:, kc * BLOC : (kc + 1) * BLOC]
                    )
            if not last:
                acc_step(xn, s)
            xp, xc, xn = xc, xn, xp

        # ---- store (host un-transposes) ----
        nc.sync.dma_start(
            outT[:].rearrange("(k p) i -> p k i", p=128), v3(acc[:])
        )

    nc.compile()
    return nc


_NC_CACHE = {}


def _get_nc():
    if "nc" not in _NC_CACHE:
        _NC_CACHE["nc"] = _build_nc()
    return _NC_CACHE["nc"]


def build_in_maps(t, x, mean0, cov0):
    t = np.ascontiguousarray(t, np.float32)
    x = np.ascontiguousarray(x, np.float32)
    mean0 = np.ascontiguousarray(mean0, np.float32)
    cov0 = np.ascontiguousarray(cov0, np.float32)
    lam, dmat, ones1, ones64, eye = _host_constants()
    mean_pk = np.ascontiguousarray(mean0.reshape(KC, 128).T)
    in_maps = []
    for i in range(NCORES):
        sl = slice(i * BLOC, (i + 1) * BLOC)
        in_maps.append(
            {
                "t_row": t[sl].reshape(1, BLOC).copy(),
                "xT": np.ascontiguousarray(x[sl].T),
                "mean_pk": mean_pk,
                "cov0": cov0,
                "lam": lam,
                "dmat": dmat,
                "ones1": ones1,
                "ones64": ones64,
                "eye": eye,
            }
        )
    return in_maps


def gather(results):
    out = np.empty((B, D), np.float32)
    for i in range(NCORES):
        out[i * BLOC : (i + 1) * BLOC, :] = results[i]["outT"].T
    return out


def kernel(t, x, mean0, cov0):
    nc = _get_nc()
    in_maps = build_in_maps(t, x, mean0, cov0)
    res = run_bass_kernel_spmd(nc, in_maps, core_ids=list(range(NCORES)))
    return gather(res.results)


# revision 12
# speedup vs baseline: 1.0201x; 1.0201x over previous
"""Analytic Gaussian VP score on 8 TRN2 NeuronCores.

Math: per sample i, score_i = -Sigma_i^{-1} (x_i - a_i*mean0) with
Sigma_i = a_i^2*cov0 + s_i^2*I.  All Sigma_i are shifted/scaled versions of
one shared matrix, so instead of 128 per-sample Choleskys we apply a
per-sample degree-NK Chebyshev polynomial of cov0:

    score_i = -sum_k c_{i,k} T_k(Mt) u_i,   Mt = (cov0 - MID*I)/HALF

The coefficients c_{i,k} are computed ON DEVICE from t_i (ScalarE exp +
DVE + small PE matmuls).  The T_k are generated four at a time: with
T4 = T_4(Mt) built once on device (two matrix squarings), the identity
T_{k+4} = 2*T4*T_k - T_{k-4} advances four independent chains per batched
matmul, so the TensorEngine runs [128,128]-weight matmuls with a 64-wide
moving operand (the 4*16 chain/sample columns) instead of a pathological
16-wide one.  A basis scaling gamma = HALF/2 folds all affine constants
into the stored matrices/coefficients; spectral bounds [L, U] hold
structurally (cov0 = PSD + 0.1*I, Marchenko-Pastur bulk < 4.1).

Sharding: pure data parallelism - mean0/cov0 replicated, the 128 (t, x)
pairs split 16 per core, no collectives.  State is kept transposed
([feature, (chunk, chain, sample)] = [128 x 256]) so the matmul is
cov0-stationary (symmetric blocks, no transposes anywhere) and all
elementwise ops run on fully-occupied 128-partition tiles.  float32r
(fp32 storage, fp22 multiply) everywhere on the matmul path; validated
rel err ~1e-3 vs the 2e-2 gate.
"""

import numpy as np

try:
    import concourse.bass as bass
except ImportError:  # fresh grading dir: point at the staged repo
    import sys

    for _p in ("/opt/trn_rl_repo", "/root/.axon_site/_ro/trn_rl_repo"):
        if _p not in sys.path:
            sys.path.insert(0, _p)
    import concourse.bass as bass

from contextlib import ExitStack

import concourse.tile as tile
from concourse import bacc, mybir
from concourse.tile import ScopedClock


def _lean_drain_and_barrier(self, tick_clock, wait_clock):
    """Tile end-sequence without per-semaphore end-clears.

    Bass clears the whole kernel semaphore range at NEFF entry, so for a
    single-TileContext kernel the end-of-kernel clear_and_free pass (~50
    sems x 5 engines of EVENT_SEMAPHORE ops + a second all-engine barrier,
    ~8us on silicon) is redundant.  Keep the drain (fences DMA queues) and
    one all-engine barrier.
    """
    drain_inst = self.nc.sync.drain()
    wait_clock.add_sem_waits(
        drain_inst.ins, ScopedClock({None: tick_clock.global_clock})
    )
    self.nc.all_engine_barrier()
    popped = self.nc._tile_sem_poison_stack.pop()
    assert popped is self._sem_poison
from concourse.bass_utils import run_bass_kernel_spmd

F32 = mybir.dt.float32
F32R = mybir.dt.float32r
BF16 = mybir.dt.bfloat16
AL = mybir.AluOpType
AX = mybir.AxisListType

B, D = 128, 512
NCORES = 8
BLOC = B // NCORES  # 16 samples per core
KC = D // 128  # 4 partition chunks of the feature dim
NCH = 4  # Chebyshev chains advanced per step
W = NCH * BLOC  # 64: moving-operand width of the main matmuls

# Chebyshev setup (input-independent constants)
L_BND, U_BND = 0.0995, 4.10
NN = 64  # interpolation nodes
NK = 19  # polynomial degree; NK+1 = 20 coefficients = 4 chains x 5 steps
NSTEP = (NK + 1) // 4 - 1  # T4-steps (first one special)
MID = (U_BND + L_BND) / 2.0
HALF = (U_BND - L_BND) / 2.0
GAMMA = HALF / 2.0  # basis scaling; makes the matmuls consume raw matrices
G2 = GAMMA * GAMMA
G8 = GAMMA**8
HALF2 = HALF * HALF
T4DIAG = HALF**4 / 8.0
BETA_MIN, BETA_MAX = 0.1, 20.0


def _host_constants():
    j = np.arange(NN)
    th = np.pi * (j + 0.5) / NN
    lam = (MID + HALF * np.cos(th)).astype(np.float32).reshape(NN, 1)
    k = np.arange(NK + 1)
    dm = (2.0 / NN) * np.cos(k[None, :] * th[:, None])
    dm[:, 0] *= 0.5
    dm = (-dm) * (1.0 / np.float64(GAMMA)) ** k[None, :]  # fold -1, gamma^-k
    dmat = dm.astype(np.float32)
    ones1 = np.ones((1, 128), np.float32)
    ones64 = np.ones((NN, 128), np.float32)
    eye = np.eye(128, dtype=np.float32)
    return lam, dmat, ones1, ones64, eye


def _build_nc():
    nc = bacc.Bacc()
    t_row = nc.declare_dram_parameter("t_row", [1, BLOC], F32, isOutput=False)
    xT = nc.declare_dram_parameter("xT", [D, BLOC], F32, isOutput=False)
    mean_pk = nc.declare_dram_parameter("mean_pk", [128, KC], F32, isOutput=False)
    cov0 = nc.declare_dram_parameter("cov0", [D, D], F32R, isOutput=False)
    lam = nc.declare_dram_parameter("lam", [NN, 1], F32, isOutput=False)
    dmat = nc.declare_dram_parameter("dmat", [NN, NK + 1], F32, isOutput=False)
    ones1 = nc.declare_dram_parameter("ones1", [1, 128], F32, isOutput=False)
    ones64 = nc.declare_dram_parameter("ones64", [NN, 128], F32, isOutput=False)
    eye = nc.declare_dram_parameter("eye", [128, 128], F32, isOutput=False)
    outT = nc.declare_dram_parameter("outT", [D, BLOC], F32, isOutput=True)

    with ExitStack() as ctx:
        tc = ctx.enter_context(tile.TileContext(nc))
        tc._drain_and_barrier = _lean_drain_and_barrier.__get__(tc)
        const = ctx.enter_context(tc.tile_pool(name="const", bufs=1))
        state = ctx.enter_context(tc.tile_pool(name="state", bufs=1))
        work = ctx.enter_context(tc.tile_pool(name="work", bufs=2))
        ps_mm = ctx.enter_context(tc.tile_pool(name="ps_mm", bufs=2, space="PSUM"))
        ps_mv = ctx.enter_context(tc.tile_pool(name="ps_mv", bufs=2, space="PSUM"))
        ps_one = ctx.enter_context(tc.tile_pool(name="ps_one", bufs=1, space="PSUM"))

        # ---- PE warm-up: memset a scratch tile (no DMA dependency), then
        # dummy fp32 matmuls so the HAM clock gate is released before the
        # real matmuls start
        warm_sb = const.tile([128, 128], F32, tag="warm_sb")
        nc.gpsimd.memset(warm_sb[:], 1.0)
        warm_ps = ps_one.tile([128, 128], F32, tag="warm")
        for _ in range(8):
            nc.tensor.matmul(warm_ps[:], warm_sb[:], warm_sb[:])
        # ---- loads (spread across engine DGE queues) ----
        eye_sb = const.tile([128, 128], F32, tag="eye")
        nc.gpsimd.dma_start(eye_sb[:], eye[:])
        cov_sb = []
        dma_engs = [nc.sync, nc.scalar, nc.gpsimd]
        for kc in range(KC):
            ct = const.tile([128, D], F32R, tag=f"cov{kc}", name=f"cov{kc}")
            for h in range(2):
                dma_engs[(2 * kc + h) % 3].dma_start(
                    ct[:, h * 256 : (h + 1) * 256],
                    cov0[kc * 128 : (kc + 1) * 128, h * 256 : (h + 1) * 256],
                )
            cov_sb.append(ct)
        xhat = state.tile([128, KC * BLOC], F32, tag="xhat")
        nc.scalar.dma_start(
            xhat[:].rearrange("p (k i) -> p k i", k=KC),
            xT[:].rearrange("(k p) i -> p k i", p=128),
        )
        mhat = const.tile([128, KC], F32, tag="mhat")
        nc.scalar.dma_start(mhat[:], mean_pk[:])
        trow = const.tile([1, BLOC], F32, tag="trow")
        nc.gpsimd.dma_start(trow[:], t_row[:])
        lam_sb = const.tile([NN, 1], F32, tag="lam")
        nc.gpsimd.dma_start(lam_sb[:], lam[:])
        dmat_sb = const.tile([NN, NK + 1], F32, tag="dmat")
        nc.sync.dma_start(dmat_sb[:], dmat[:])
        ones1_sb = const.tile([1, 128], F32, tag="ones1")
        nc.scalar.dma_start(ones1_sb[:], ones1[:])
        ones64_sb = const.tile([NN, 128], F32, tag="ones64")
        nc.sync.dma_start(ones64_sb[:], ones64[:])

        # ---- per-sample scalars from t ----
        u9 = const.tile([1, BLOC], F32, tag="u9")
        nc.vector.tensor_scalar(u9[:], trow[:], 9.95, 0.1, AL.mult, AL.add)
        ib = const.tile([1, BLOC], F32, tag="ib")
        nc.vector.tensor_mul(ib[:], u9[:], trow[:])
        a_row = const.tile([1, BLOC], F32, tag="a_row")
        nc.scalar.activation(
            a_row[:], ib[:], mybir.ActivationFunctionType.Exp, scale=-0.5
        )
        abc = const.tile([1, 3 * BLOC], F32, tag="abc")  # [a | a^2 | s^2]
        nc.vector.tensor_copy(abc[:, 0:BLOC], a_row[:])
        nc.vector.tensor_mul(abc[:, BLOC : 2 * BLOC], a_row[:], a_row[:])
        nc.vector.tensor_scalar(
            abc[:, 2 * BLOC :], abc[:, BLOC : 2 * BLOC], -1.0, 1.0, AL.mult, AL.add
        )
        nc.vector.tensor_scalar_max(abc[:, 2 * BLOC :], abc[:, 2 * BLOC :], 1e-12)

        # broadcast [a | a^2 | s^2] down all 128 partitions via a K=1 matmul
        rep_ps = ps_one.tile([128, 3 * BLOC], F32, tag="rep")
        nc.tensor.matmul(rep_ps[:], ones1_sb[:], abc[:])
        rep = const.tile([128, 3 * BLOC], F32, tag="rep_sb")
        nc.scalar.copy(rep[:], rep_ps[:])
        a_rep = rep[:, 0:BLOC]
        a2_rep = rep[:, BLOC : 2 * BLOC]
        s2_rep = rep[:, 2 * BLOC : 3 * BLOC]

        # ---- Chebyshev coefficients on device ----
        q = const.tile([NN, BLOC], F32, tag="q")
        nc.vector.scalar_tensor_tensor(
            q[:], a2_rep[0:NN, :], lam_sb[:, 0:1], s2_rep[0:NN, :], AL.mult, AL.add
        )
        fhat = const.tile([NN, BLOC], F32, tag="fhat")
        nc.vector.reciprocal(fhat[:], q[:])
        rhs_t = const.tile([NN, (NK + 1) * BLOC], F32, tag="rhs_t")
        nc.vector.tensor_mul(
            rhs_t[:].rearrange("p (k i) -> p k i", k=NK + 1),
            fhat[:].unsqueeze(1).broadcast_to((NN, NK + 1, BLOC)),
            dmat_sb[:].unsqueeze(2).broadcast_to((NN, NK + 1, BLOC)),
        )
        c_ps = ps_one.tile([128, (NK + 1) * BLOC], F32, tag="cps")
        nc.tensor.matmul(c_ps[:], ones64_sb[:], rhs_t[:])
        c_sb = const.tile([128, (NK + 1) * BLOC], F32, tag="c_sb")
        nc.scalar.copy(c_sb[:], c_ps[:])

        def cstep(s):
            """coefficients for step s: [128, (chain, sample)] bcast over kc."""
            return (
                c_sb[:, s * W : (s + 1) * W]
                .unsqueeze(1)
                .broadcast_to((128, KC, W))
            )

        # ---- Btil = C^2 - 2*MID*C + MID^2*I  (stored as 4 row-chunk tiles) --
        btil = [
            const.tile([128, D], F32R, tag=f"btil{r}", name=f"btil{r}")
            for r in range(KC)
        ]
        for r in range(KC):
            c2 = ps_mm.tile([128, D], F32, tag="mm", name="c2")
            for kc in range(KC):
                nc.tensor.matmul(
                    c2[:],
                    cov_sb[kc][:, r * 128 : (r + 1) * 128],
                    cov_sb[kc][:],
                    start=(kc == 0),
                    stop=(kc == KC - 1),
                )
            nc.vector.scalar_tensor_tensor(
                btil[r][:], cov_sb[r][:], -2.0 * MID, c2[:], AL.mult, AL.add
            )
            nc.vector.scalar_tensor_tensor(
                btil[r][:, r * 128 : (r + 1) * 128],
                eye_sb[:],
                MID * MID,
                btil[r][:, r * 128 : (r + 1) * 128],
                AL.mult,
                AL.add,
            )

        # ---- T4h = Btil^2 - HALF^2*Btil + (HALF^4/8)*I  (= 2*gamma^4*T_4(Mt))
        t4 = [
            const.tile([128, D], F32R, tag=f"t4{r}", name=f"t4{r}")
            for r in range(KC)
        ]
        for r in range(KC):
            b2 = ps_mm.tile([128, D], F32, tag="mm", name="b2")
            for kc in range(KC):
                nc.tensor.matmul(
                    b2[:],
                    btil[kc][:, r * 128 : (r + 1) * 128],
                    btil[kc][:],
                    start=(kc == 0),
                    stop=(kc == KC - 1),
                )
            nc.vector.scalar_tensor_tensor(
                t4[r][:], btil[r][:], -HALF2, b2[:], AL.mult, AL.add
            )
            nc.vector.scalar_tensor_tensor(
                t4[r][:, r * 128 : (r + 1) * 128],
                eye_sb[:],
                T4DIAG,
                t4[r][:, r * 128 : (r + 1) * 128],
                AL.mult,
                AL.add,
            )

        # ---- split T4h into bf16 hi + lo (weights at ~2^-16 precision) ----
        thi = [
            const.tile([128, D], BF16, tag=f"thi{r}", name=f"thi{r}")
            for r in range(KC)
        ]
        tlo = [
            const.tile([128, D], BF16, tag=f"tlo{r}", name=f"tlo{r}")
            for r in range(KC)
        ]
        for r in range(KC):
            nc.scalar.copy(thi[r][:], t4[r][:].bitcast(F32))
            nc.vector.tensor_sub(tlo[r][:], t4[r][:].bitcast(F32), thi[r][:])

        # ---- state: X [128, (kc, chain, sample)], bf16 ----
        xs = [
            state.tile([128, KC * W], BF16, tag=f"X{i}", name=f"X{i}")
            for i in range(3)
        ]
        y01 = state.tile([128, KC * 2 * BLOC], F32R, tag="y01")  # f32 Y0,Y1
        acc = state.tile([128, KC * BLOC], F32, tag="acc")

        def chain(st, r):
            """[128, kc, BLOC] view of chain r of state tile st."""
            return st[:].rearrange("p (k r i) -> p k r i", k=KC, r=NCH)[:, :, r, :]

        def v3(ap):
            return ap.rearrange("p (k i) -> p k i", k=KC)

        def matvec(dst_ps, mats, src):
            """dst_ps[:, mc*16:...] += mats^T-block @ src ([128,16] slices)."""
            for mc in range(KC):
                for kc in range(KC):
                    nc.tensor.matmul(
                        dst_ps[:, mc * BLOC : (mc + 1) * BLOC],
                        mats[kc][:, mc * 128 : (mc + 1) * 128],
                        src[:, kc, :].bitcast(F32R),
                        start=(kc == 0),
                        stop=(kc == KC - 1),
                    )

        def matstep(dst_ps, st):
            """dst_ps[:, mc*W:...] += T4h-block @ st, bf16 hi/lo weights."""
            for mc in range(KC):
                first = True
                for kc in range(KC):
                    for mats in (thi, tlo):
                        nc.tensor.matmul(
                            dst_ps[:, mc * W : (mc + 1) * W],
                            mats[kc][:, mc * 128 : (mc + 1) * 128],
                            st[:, kc * W : (kc + 1) * W],
                            start=first,
                            stop=(kc == KC - 1 and mats is tlo),
                        )
                        first = False

        # ---- init: Y0 = u = x - a*mean0 (f32 scratch y01 + bf16 chain) ----
        x0 = xs[0]
        y01v = y01[:].rearrange("p (k r i) -> p k r i", k=KC, r=2)
        w1 = work.tile([128, KC * BLOC], F32, tag="w1")
        nc.vector.tensor_mul(
            v3(w1[:]),
            a_rep.unsqueeze(1).broadcast_to((128, KC, BLOC)),
            mhat[:].unsqueeze(2).broadcast_to((128, KC, BLOC)),
        )
        nc.vector.tensor_sub(y01v[:, :, 0, :], v3(xhat[:]), v3(w1[:]))
        nc.scalar.copy(chain(x0, 0), y01v[:, :, 0, :])

        # Y1 = 0.5*C@Y0 - (MID/2)*Y0
        p1 = ps_mv.tile([128, KC * BLOC], F32, tag="pmv", name="p1")
        matvec(p1, cov_sb, y01v[:, :, 0, :])
        w2 = work.tile([128, KC * BLOC], F32, tag="w2")
        nc.vector.tensor_scalar_mul(v3(w2[:]), y01v[:, :, 0, :], -MID / 2.0)
        nc.vector.scalar_tensor_tensor(
            y01v[:, :, 1, :], v3(p1[:]), 0.5, v3(w2[:]), AL.mult, AL.add
        )
        nc.scalar.copy(chain(x0, 1), y01v[:, :, 1, :])

        # one T2-step: P = Btil@[Y0|Y1]; Y2 = 0.5*P0 - g2*Y0; Y3 = P1 - 3*g2*Y1
        p2 = ps_mv.tile([128, KC * 2 * BLOC], F32, tag="pmv", name="p2")
        for mc in range(KC):
            for kc in range(KC):
                nc.tensor.matmul(
                    p2[:, mc * 2 * BLOC : (mc + 1) * 2 * BLOC],
                    btil[kc][:, mc * 128 : (mc + 1) * 128],
                    y01[:, kc * 2 * BLOC : (kc + 1) * 2 * BLOC].bitcast(F32R),
                    start=(kc == 0),
                    stop=(kc == KC - 1),
                )
        p2v = p2[:].rearrange("p (k r i) -> p k r i", k=KC, r=2)
        w3 = work.tile([128, KC * BLOC], F32, tag="w2", name="w3")
        nc.vector.tensor_scalar_mul(v3(w3[:]), y01v[:, :, 0, :], -G2)
        nc.vector.scalar_tensor_tensor(
            chain(x0, 2), p2v[:, :, 0, :], 0.5, v3(w3[:]), AL.mult, AL.add
        )
        nc.vector.scalar_tensor_tensor(
            chain(x0, 3), y01v[:, :, 1, :], -3.0 * G2, p2v[:, :, 1, :],
            AL.mult, AL.add,
        )

        def acc_step(st, s, first=False):
            """acc[:, kc, i] += sum_r c[4s+r, i] * st[:, kc, r, i]."""
            mt = work.tile([128, KC * W], F32, tag="mt")
            nc.vector.tensor_mul(
                mt[:].rearrange("p (k w) -> p k w", k=KC),
                st[:].rearrange("p (k w) -> p k w", k=KC),
                cstep(s),
            )
            red = mt[:].rearrange("p (k r i) -> p k i r", k=KC, r=NCH)
            if first:
                nc.vector.tensor_reduce(v3(acc[:]), red, AX.X, AL.add)
            else:
                rt = work.tile([128, KC * BLOC], F32, tag="rt")
                nc.vector.tensor_reduce(v3(rt[:]), red, AX.X, AL.add)
                nc.vector.tensor_add(acc[:], acc[:], rt[:])

        acc_step(x0, 0, first=True)

        # ---- step 1 (special): X1[r] = T4h@X0[r] - g^{2r}*Y_{4-r}; r=0 halved
        x1 = xs[1]
        z = ps_mm.tile([128, KC * W], F32, tag="mm", name="z1")
        matstep(z, x0[:])
        zv = z[:].rearrange("p (k r i) -> p k r i", k=KC, r=NCH)
        nc.vector.tensor_scalar_mul(chain(x1, 0), zv[:, :, 0, :], 0.5)
        for r in (1, 2, 3):
            nc.vector.scalar_tensor_tensor(
                chain(x1, r),
                chain(x0, NCH - r),
                -(GAMMA ** (2 * r)),
                zv[:, :, r, :],
                AL.mult,
                AL.add,
            )
        acc_step(x1, 1)

        # ---- steps 2..NSTEP: Xn = T4h@Xc - gamma^8*Xp ----
        xp, xc, xn = xs
        outv = outT[:].rearrange("(k p) i -> p k i", p=128)
        dma_engs2 = [nc.sync, nc.scalar, nc.gpsimd, nc.sync]
        for s in range(2, NSTEP + 1):
            P = ps_mm.tile([128, KC * W], F32, tag="mm", name=f"P{s}")
            matstep(P, xc[:])
            last = s == NSTEP
            for kc in range(KC):  # chunked: next step's kc=0 mm starts early
                sl = slice(kc * W, (kc + 1) * W)
                nc.vector.scalar_tensor_tensor(
                    xn[:, sl], xp[:, sl], -G8, P[:, sl], AL.mult, AL.add
                )
                if last:
                    # finish acc for this chunk and ship it out
                    mt = work.tile([128, W], F32, tag="mtc", name=f"mtc{kc}")
                    nc.vector.tensor_mul(
                        mt[:], xn[:, sl], c_sb[:, s * W : (s + 1) * W]
                    )
                    rt = work.tile([128, BLOC], F32, tag="rtc", name=f"rtc{kc}")
                    nc.vector.tensor_reduce(
                        rt[:].unsqueeze(1),
                        mt[:].rearrange("p (r i) -> p i r", r=NCH),
                        AX.X,
                        AL.add,
                    )
                    nc.vector.tensor_add(
                        acc[:, kc * BLOC : (kc + 1) * BLOC],
                        acc[:, kc * BLOC : (kc + 1) * BLOC],
                        rt[:],
                    )
                    dma_engs2[kc].dma_start(
                        outv[:, kc, :], acc[:, kc * BLOC : (kc + 1) * BLOC]
                    )
            if not last:
                acc_step(xn, s)
            xp, xc, xn = xc, xn, xp

        # ---- store (host un-transposes) ----
        nc.sync.dma_start(
            outT[:].rearrange("(k p) i -> p k i", p=128), v3(acc[:])
        )

    nc.compile()
    return nc


_NC_CACHE = {}


def _get_nc():
    if "nc" not in _NC_CACHE:
        _NC_CACHE["nc"] = _build_nc()
    return _NC_CACHE["nc"]


def build_in_maps(t, x, mean0, cov0):
    t = np.ascontiguousarray(t, np.float32)
    x = np.ascontiguousarray(x, np.float32)
    mean0 = np.ascontiguousarray(mean0, np.float32)
    cov0 = np.ascontiguousarray(cov0, np.float32)
    lam, dmat, ones1, ones64, eye = _host_constants()
    mean_pk = np.ascontiguousarray(mean0.reshape(KC, 128).T)
    in_maps = []
    for i in range(NCORES):
        sl = slice(i * BLOC, (i + 1) * BLOC)
        in_maps.append(
            {
                "t_row": t[sl].reshape(1, BLOC).copy(),
                "xT": np.ascontiguousarray(x[sl].T),
                "mean_pk": mean_pk,
                "cov0": cov0,
                "lam": lam,
                "dmat": dmat,
                "ones1": ones1,
                "ones64": ones64,
                "eye": eye,
            }
        )
    return in_maps


def gather(results):
    out = np.empty((B, D), np.float32)
    for i in range(NCORES):
        out[i * BLOC : (i + 1) * BLOC, :] = results[i]["outT"].T
    return out


def kernel(t, x, mean0, cov0):
    nc = _get_nc()
    in_maps = build_in_maps(t, x, mean0, cov0)
    res = run_bass_kernel_spmd(nc, in_maps, core_ids=list(range(NCORES)))
    return gather(res.results)
